# revision 23
# baseline (speedup 1.0000x reference)
"""SAGAN-style attention block (B=16, C=64, H=W=64) on 8 trn2 NeuronCores.

Data-parallel over batch: 2 samples per core.  Per sample:
    g/phi conv -> 2x2 maxpool           (PE + DVE)
    phi~ = Wt^T @ phi                   (PE)   [folds theta away: scoresT = phi~^T x]
    scoresT[s,t] = phi~^T x  (s-chunks of 128 on partitions, t on free)
    expT = exp(scoresT)                 (ACT, no max-subtraction: |scores| < ~6)
    o_un[c,t], denom[t] = [g;1]^T-weighted matmul over s   (PE, ones-row trick)
    o_norm = o_un * (1/denom)           (DVE recip approx + GPSIMD partition bcast)
    y = (gamma*Wo) @ o_norm             (PE)
    out = y + x                         (DVE, fp32 residual)
"""

import numpy as np
import ml_dtypes

import concourse.bass as bass
import concourse.bacc as bacc
import concourse.tile as tile
from concourse import mybir
from concourse.bass_utils import run_bass_kernel_spmd

FP32 = mybir.dt.float32
BF16 = mybir.dt.bfloat16
I16 = mybir.dt.int16
SCH_A = float(128.0 / np.log(2.0))   # Schraudolph bf16: round(A*x+B) -> bf16 bits
SCH_B = 16256.0 - 5.6
ts = bass.ts

C = 64
HW = 4096          # 64*64
S = 1024           # pooled spatial (32*32)
NSAMP = 2          # samples per core
NCHUNK = 8         # both t-chunks (512 wide) and conv chunks
TC = 512           # t-chunk width
SC = 128           # s-chunk width


def build_nc(n_samples: int = NSAMP, repeat: int = 1) -> bass.Bass:
    nc = bacc.Bacc("TRN2", target_bir_lowering=False, debug=False)

    x_bf = nc.dram_tensor("x_bf", [n_samples, C, HW], BF16, kind="ExternalInput").ap()
    x_f32 = nc.dram_tensor("x_f32", [n_samples, C, HW], FP32, kind="ExternalInput").ap()
    # convA weights: columns 0-31 -> g channels (Wg), columns 32-39 -> phi (Wp)
    w_gp = nc.dram_tensor("w_gp", [C, 40], BF16, kind="ExternalInput").ap()
    # phi~ weights: rows 32-39 hold [Wt | Wt] [8, 128]; rows 0-31 zeros
    w_tt = nc.dram_tensor("w_tt", [40, 2 * C], BF16, kind="ExternalInput").ap()
    # final conv weights: (gamma*Wo)^T  [32, 64]
    w_o = nc.dram_tensor("w_o", [32, C], BF16, kind="ExternalInput").ap()
    ident = nc.dram_tensor("ident", [32, 32], BF16, kind="ExternalInput").ap()
    out = nc.dram_tensor("out", [n_samples, C, HW], FP32, kind="ExternalOutput").ap()

    with tile.TileContext(nc) as tc:
        for _ in range(repeat):
            _body(tc, n_samples, x_bf, x_f32, w_gp, w_tt, w_o, ident, out)
    nc.compile()
    return nc


def _body(tc, n_samples, x_bf, x_f32, w_gp, w_tt, w_o, ident, out):
    nc = tc.nc
    from contextlib import ExitStack

    with ExitStack() as ctx:
        consts = ctx.enter_context(tc.tile_pool(name="consts", bufs=1))
        xpool = ctx.enter_context(tc.tile_pool(name="xpool", bufs=2))
        mid = ctx.enter_context(tc.tile_pool(name="mid", bufs=2))
        expp = ctx.enter_context(tc.tile_pool(name="expp", bufs=4))
        smal = ctx.enter_context(tc.tile_pool(name="smal", bufs=3))
        outp = ctx.enter_context(tc.tile_pool(name="outp", bufs=2))
        ps_conv = ctx.enter_context(tc.tile_pool(name="ps_conv", bufs=1, space="PSUM"))
        ps_scorA = ctx.enter_context(tc.tile_pool(name="ps_scorA", bufs=1, space="PSUM"))
        ps_scorB = ctx.enter_context(tc.tile_pool(name="ps_scorB", bufs=1, space="PSUM"))
        ps_oacc = ctx.enter_context(tc.tile_pool(name="ps_oacc", bufs=2, space="PSUM"))
        ps_fin = ctx.enter_context(tc.tile_pool(name="ps_fin", bufs=1, space="PSUM"))

        wgp_sb = consts.tile([C, 40], BF16)
        nc.sync.dma_start(wgp_sb[:], w_gp[:])
        wtt_sb = consts.tile([40, 2 * C], BF16)
        nc.sync.dma_start(wtt_sb[:], w_tt[:])
        wo_sb = consts.tile([32, C], BF16)
        nc.sync.dma_start(wo_sb[:], w_o[:])
        id_sb = consts.tile([32, 32], BF16)
        nc.sync.dma_start(id_sb[:], ident[:])

        # ---- setup phase for every sample first (overlaps with attention of
        # earlier samples via scheduler priorities) ----------------------------
        setup = []
        for i in range(n_samples):
            xb = xpool.tile([2 * C, HW], BF16, tag="xb")
            xf = xpool.tile([C, HW], FP32, tag="xf")
            pooled = mid.tile([40, S], BF16, tag="pooled")
            phi2 = mid.tile([2 * C, S], BF16, tag="phi2")
            gT = mid.tile([SC, 33 * NCHUNK], BF16, tag="gT")

            # convA (g + phi) + 2x2 maxpool, per 512-col chunk; x replicated
            # to partitions 64-127 for 2-way row-packed scores matmuls
            for c in range(NCHUNK):
                nc.sync.dma_start(xb[0:C, ts(c, TC)], x_bf[i][:, ts(c, TC)])
                nc.sync.dma_start(xb[C : 2 * C, ts(c, TC)], x_bf[i][:, ts(c, TC)])
                nc.sync.dma_start(xf[:, ts(c, TC)], x_f32[i][:, ts(c, TC)])
                pa = ps_conv.tile([40, TC], FP32, tag="conv")
                nc.tensor.matmul(pa[:], wgp_sb[:], xb[0:C, ts(c, TC)])
                v = pa[:].rearrange("p (h eh w ew) -> p h w eh ew", h=4, eh=2, w=32, ew=2)
                pv = pooled[:, ts(c, SC)].rearrange("p (h w) -> p h w", h=4, w=32)
                nc.vector.tensor_reduce(
                    pv, v, axis=mybir.AxisListType.XY, op=mybir.AluOpType.max,
                    opt_input=False,
                )
                # phi~ for this s-chunk: [128, 128] (both replicas at once)
                ppt = ps_conv.tile([2 * C, SC], FP32, tag="conv")
                nc.tensor.matmul(
                    ppt[:], wtt_sb[32:40, :], pooled[32:40, ts(c, SC)]
                )
                nc.vector.tensor_copy(phi2[:, ts(c, SC)], ppt[:])

            # g'^T chunks [128, 33] with ones column
            ones_view = gT[:].rearrange("p (k c) -> p k c", k=NCHUNK, c=33)
            nc.vector.memset(ones_view[:, :, 32:33], 1.0)
            for k in range(NCHUNK):
                pt = ps_conv.tile([SC, 32], BF16, tag="conv")
                nc.tensor.transpose(pt[:], pooled[0:32, ts(k, SC)], id_sb[:])
                nc.vector.tensor_copy(gT[:, 33 * k : 33 * k + 32], pt[:])
            setup.append((xb, xf, phi2, gT))

        for i in range(n_samples):
            xb, xf, phi2, gT = setup[i]
            # ---- attention main loop over t-chunks ---------------------------
            o_norm = mid.tile([32, HW], BF16, tag="o_norm")
            out_sb = outp.tile([C, HW], FP32, tag="out_sb")
            for t in range(NCHUNK):
                po = ps_oacc.tile([33, TC], FP32, tag="oacc")
                for q in range(4):  # four groups of 2 s-chunks, alternating pools
                    pool_q = ps_scorA if q % 2 == 0 else ps_scorB
                    pscr = pool_q.tile([SC, 2 * TC], FP32, tag="scor")
                    # 2-way row-packed: s-chunk 2q on rows 0-63, 2q+1 on 64-127
                    nc.tensor.matmul(
                        pscr[:, ts(0, TC)],
                        phi2[0:C, ts(2 * q, SC)],
                        xb[0:C, ts(t, TC)],
                        tile_position=(0, 0),
                    )
                    nc.tensor.matmul(
                        pscr[:, ts(1, TC)],
                        phi2[C : 2 * C, ts(2 * q + 1, SC)],
                        xb[C : 2 * C, ts(t, TC)],
                        tile_position=(64, 0),
                    )
                    unit = (i * NCHUNK + t) * 4 + q
                    if unit % 7 == 3:
                        # Schraudolph exp on DVE: bf16 bit pattern via int16
                        e16 = expp.tile([SC, 2 * TC], I16, tag="expT")
                        nc.vector.tensor_scalar(
                            e16[:], pscr[:], SCH_A, SCH_B,
                            mybir.AluOpType.mult, mybir.AluOpType.add,
                        )
                        expT = e16[:].bitcast(BF16)
                    else:
                        et = expp.tile([SC, 2 * TC], BF16, tag="expT")
                        nc.scalar.activation(
                            et[:], pscr[:], mybir.ActivationFunctionType.Exp
                        )
                        expT = et[:]
                    for j in range(2):
                        sc = 2 * q + j
                        nc.tensor.matmul(
                            po[:],
                            gT[:, 33 * sc : 33 * sc + 33],
                            expT[:, ts(j, TC)],
                            start=(sc == 0),
                            stop=(sc == 7),
                        )
                # reciprocal of denominator row: custom-DVE + gpsimd bcast only
                # work at base partition 0 on HW, so recip all 33 lanes, DMA
                # lane 32 -> lane 0, then broadcast.
                rrow = smal.tile([33, TC], FP32, tag="rrow")
                nc.vector.reciprocal_approx_fast(rrow[:], po[:])
                r0 = smal.tile([1, TC], FP32, tag="r0")
                nc.gpsimd.dma_start(r0[:], rrow[32:33, :])
                rb = smal.tile([32, TC], FP32, tag="rb")
                nc.gpsimd.partition_broadcast(rb[:], r0[:])
                nc.vector.tensor_mul(o_norm[:, ts(t, TC)], po[0:32, :], rb[:])
                # final conv + fp32 residual
                py = ps_fin.tile([C, TC], FP32, tag="fin")
                nc.tensor.matmul(py[:], wo_sb[:], o_norm[:, ts(t, TC)])
                nc.vector.tensor_add(out_sb[:, ts(t, TC)], py[:], xf[:, ts(t, TC)])
                nc.sync.dma_start(out[i][:, ts(t, TC)], out_sb[:, ts(t, TC)])


# ---------------------------------------------------------------------------
# host-side driver
# ---------------------------------------------------------------------------

def _prep_consts(Wt, Wp, Wg, Wo, gamma):
    bf = ml_dtypes.bfloat16
    w_gp = np.zeros((C, 40), np.float32)
    w_gp[:, 0:32] = Wg.T
    w_gp[:, 32:40] = Wp.T
    w_tt = np.zeros((40, 2 * C), np.float32)
    w_tt[32:40, 0:C] = Wt
    w_tt[32:40, C : 2 * C] = Wt
    w_o = (np.float32(gamma) * Wo).T
    ident = np.eye(32, dtype=np.float32)
    return {
        "w_gp": w_gp.astype(bf),
        "w_tt": w_tt.astype(bf),
        "w_o": np.ascontiguousarray(w_o).astype(bf),
        "ident": ident.astype(bf),
    }


def kernel(x, Wt, Wp, Wg, Wo, gamma):
    x = np.asarray(x, dtype=np.float32)
    B = x.shape[0]
    n_cores = 8
    nper = B // n_cores
    xr = np.ascontiguousarray(x.reshape(B, C, HW))
    consts = _prep_consts(
        np.asarray(Wt, np.float32),
        np.asarray(Wp, np.float32),
        np.asarray(Wg, np.float32),
        np.asarray(Wo, np.float32),
        np.float32(gamma),
    )
    bf = ml_dtypes.bfloat16

    nc = build_nc(nper)
    in_maps = []
    for cid in range(n_cores):
        shard = xr[cid * nper : (cid + 1) * nper]
        in_maps.append(
            {
                "x_bf": shard.astype(bf),
                "x_f32": shard,
                **consts,
            }
        )
    res = run_bass_kernel_spmd(nc, in_maps, core_ids=list(range(n_cores)))
    outs = [res.results[cid]["out"] for cid in range(n_cores)]
    return np.concatenate(outs, axis=0).reshape(B, C, 64, 64)


# revision 27
# speedup vs baseline: 1.7423x; 1.7423x over previous
"""SAGAN-style attention block (B=16, C=64, H=W=64) on 8 trn2 NeuronCores.

Data-parallel over batch: 2 samples per core.  Per sample:
    g/phi conv -> 2x2 maxpool           (PE + DVE)
    phi~ = Wt^T @ phi                   (PE)   [folds theta away: scoresT = phi~^T x]
    scoresT[s,t] = phi~^T x  (s-chunks of 128 on partitions, t on free)
    expT = exp(scoresT)                 (ACT, no max-subtraction: |scores| < ~6)
    o_un[c,t], denom[t] = [g;1]^T-weighted matmul over s   (PE, ones-row trick)
    o_norm = o_un * (1/denom)           (DVE recip approx + GPSIMD partition bcast)
    y = (gamma*Wo) @ o_norm             (PE)
    out = y + x                         (DVE, fp32 residual)
"""

import numpy as np
import ml_dtypes

import concourse.bass as bass
import concourse.bacc as bacc
import concourse.tile as tile
from concourse import mybir
from concourse.bass_utils import run_bass_kernel_spmd

FP32 = mybir.dt.float32
BF16 = mybir.dt.bfloat16
I16 = mybir.dt.int16
SCH_A = float(128.0 / np.log(2.0))   # Schraudolph bf16: round(A*x+B) -> bf16 bits
SCH_B = 16256.0 - 5.6
SCHRAUDOLPH = False  # DVE-exp offload: off (ACT wasn't the span limiter)
ts = bass.ts

C = 64
HW = 4096          # 64*64
S = 1024           # pooled spatial (32*32)
NSAMP = 2          # samples per core
NCHUNK = 8         # both t-chunks (512 wide) and conv chunks
TC = 512           # t-chunk width
SC = 128           # s-chunk width


def build_nc(n_samples: int = NSAMP, repeat: int = 1) -> bass.Bass:
    nc = bacc.Bacc("TRN2", target_bir_lowering=False, debug=False)

    x_bf = nc.dram_tensor("x_bf", [n_samples, C, HW], BF16, kind="ExternalInput").ap()
    x_f32 = nc.dram_tensor("x_f32", [n_samples, C, HW], FP32, kind="ExternalInput").ap()
    # convA weights: columns 0-31 -> g channels (Wg), columns 32-39 -> phi (Wp)
    w_gp = nc.dram_tensor("w_gp", [C, 40], BF16, kind="ExternalInput").ap()
    # phi~ weights: rows 32-39 hold [Wt | Wt] [8, 128]; rows 0-31 zeros
    w_tt = nc.dram_tensor("w_tt", [40, 2 * C], BF16, kind="ExternalInput").ap()
    # final conv weights: (gamma*Wo)^T  [32, 64]
    w_o = nc.dram_tensor("w_o", [32, C], BF16, kind="ExternalInput").ap()
    ident = nc.dram_tensor("ident", [32, 32], BF16, kind="ExternalInput").ap()
    out = nc.dram_tensor("out", [n_samples, C, HW], FP32, kind="ExternalOutput").ap()

    with tile.TileContext(nc) as tc:
        for _ in range(repeat):
            _body(tc, n_samples, x_bf, x_f32, w_gp, w_tt, w_o, ident, out)
    nc.compile()
    return nc


def _body(tc, n_samples, x_bf, x_f32, w_gp, w_tt, w_o, ident, out):
    nc = tc.nc
    from contextlib import ExitStack

    with ExitStack() as ctx:
        consts = ctx.enter_context(tc.tile_pool(name="consts", bufs=1))
        xpool = ctx.enter_context(tc.tile_pool(name="xpool", bufs=2))
        mid = ctx.enter_context(tc.tile_pool(name="mid", bufs=2))
        expp = ctx.enter_context(tc.tile_pool(name="expp", bufs=6))
        smal = ctx.enter_context(tc.tile_pool(name="smal", bufs=3))
        outp = ctx.enter_context(tc.tile_pool(name="outp", bufs=2))
        ps_conv = ctx.enter_context(tc.tile_pool(name="ps_conv", bufs=1, space="PSUM"))
        ps_scorA = ctx.enter_context(tc.tile_pool(name="ps_scorA", bufs=1, space="PSUM"))
        ps_scorB = ctx.enter_context(tc.tile_pool(name="ps_scorB", bufs=1, space="PSUM"))
        ps_oacc = ctx.enter_context(tc.tile_pool(name="ps_oacc", bufs=2, space="PSUM"))
        ps_fin = ctx.enter_context(tc.tile_pool(name="ps_fin", bufs=1, space="PSUM"))

        # warm the ACT exp table set during setup (table load is ~2.7us)
        warm = consts.tile([1, 1], FP32)
        nc.vector.memset(warm[:], 0.0)
        nc.scalar.activation(warm[:], warm[:], mybir.ActivationFunctionType.Exp)

        wgp_sb = consts.tile([C, 40], BF16)
        nc.sync.dma_start(wgp_sb[:], w_gp[:])
        wtt_sb = consts.tile([40, 2 * C], BF16)
        nc.sync.dma_start(wtt_sb[:], w_tt[:])
        wo_sb = consts.tile([32, C], BF16)
        nc.sync.dma_start(wo_sb[:], w_o[:])
        id_sb = consts.tile([32, 32], BF16)
        nc.sync.dma_start(id_sb[:], ident[:])

        # ---- setup phase for every sample first (overlaps with attention of
        # earlier samples via scheduler priorities) ----------------------------
        setup = []
        for i in range(n_samples):
            xb = xpool.tile([2 * C, HW], BF16, tag="xb")
            xf = xpool.tile([C, HW], FP32, tag="xf")
            pooled = mid.tile([40, S], BF16, tag="pooled")
            phi2 = mid.tile([2 * C, S], BF16, tag="phi2")
            gT = mid.tile([SC, 33 * NCHUNK], BF16, tag="gT")

            # convA (g + phi) + 2x2 maxpool, per 512-col chunk; x replicated
            # to partitions 64-127 for 2-way row-packed scores matmuls
            for c in range(NCHUNK):
                nc.sync.dma_start(xb[0:C, ts(c, TC)], x_bf[i][:, ts(c, TC)])
                nc.sync.dma_start(xb[C : 2 * C, ts(c, TC)], x_bf[i][:, ts(c, TC)])
                nc.sync.dma_start(xf[:, ts(c, TC)], x_f32[i][:, ts(c, TC)])
                pa = ps_conv.tile([40, TC], FP32, tag="conv")
                nc.tensor.matmul(pa[:], wgp_sb[:], xb[0:C, ts(c, TC)])
                v = pa[:].rearrange("p (h eh w ew) -> p h w eh ew", h=4, eh=2, w=32, ew=2)
                pv = pooled[:, ts(c, SC)].rearrange("p (h w) -> p h w", h=4, w=32)
                nc.vector.tensor_reduce(
                    pv, v, axis=mybir.AxisListType.XY, op=mybir.AluOpType.max,
                    opt_input=False,
                )
                # phi~ for this s-chunk: [128, 128] (both replicas at once)
                ppt = ps_conv.tile([2 * C, SC], FP32, tag="conv")
                nc.tensor.matmul(
                    ppt[:], wtt_sb[32:40, :], pooled[32:40, ts(c, SC)]
                )
                nc.vector.tensor_copy(phi2[:, ts(c, SC)], ppt[:])

            # g'^T chunks [128, 33] with ones column
            ones_view = gT[:].rearrange("p (k c) -> p k c", k=NCHUNK, c=33)
            nc.vector.memset(ones_view[:, :, 32:33], 1.0)
            for k in range(NCHUNK):
                pt = ps_conv.tile([SC, 32], BF16, tag="conv")
                nc.tensor.transpose(pt[:], pooled[0:32, ts(k, SC)], id_sb[:])
                nc.vector.tensor_copy(gT[:, 33 * k : 33 * k + 32], pt[:])
            setup.append((xb, xf, phi2, gT))

        for i in range(n_samples):
            xb, xf, phi2, gT = setup[i]
            # ---- attention main loop over t-chunks ---------------------------
            o_norm = mid.tile([32, HW], BF16, tag="o_norm")
            out_sb = outp.tile([C, HW], FP32, tag="out_sb")
            for t in range(NCHUNK):
                po = ps_oacc.tile([33, TC], FP32, tag="oacc")
                for q in range(4):  # four groups of 2 s-chunks, alternating pools
                    pool_q = ps_scorA if q % 2 == 0 else ps_scorB
                    pscr = pool_q.tile([SC, 2 * TC], FP32, tag="scor")
                    # 2-way row-packed: s-chunk 2q on rows 0-63, 2q+1 on 64-127
                    nc.tensor.matmul(
                        pscr[:, ts(0, TC)],
                        phi2[0:C, ts(2 * q, SC)],
                        xb[0:C, ts(t, TC)],
                        tile_position=(0, 0),
                    )
                    nc.tensor.matmul(
                        pscr[:, ts(1, TC)],
                        phi2[C : 2 * C, ts(2 * q + 1, SC)],
                        xb[C : 2 * C, ts(t, TC)],
                        tile_position=(64, 0),
                    )
                    unit = (i * NCHUNK + t) * 4 + q
                    if SCHRAUDOLPH and unit % 7 == 3:
                        # Schraudolph exp on DVE: bf16 bit pattern via int16
                        e16 = expp.tile([SC, 2 * TC], I16, tag="expT")
                        nc.vector.tensor_scalar(
                            e16[:], pscr[:], SCH_A, SCH_B,
                            mybir.AluOpType.mult, mybir.AluOpType.add,
                        )
                        expT = e16[:].bitcast(BF16)
                    else:
                        et = expp.tile([SC, 2 * TC], BF16, tag="expT")
                        nc.scalar.activation(
                            et[:], pscr[:], mybir.ActivationFunctionType.Exp
                        )
                        expT = et[:]
                    for j in range(2):
                        sc = 2 * q + j
                        nc.tensor.matmul(
                            po[:],
                            gT[:, 33 * sc : 33 * sc + 33],
                            expT[:, ts(j, TC)],
                            start=(sc == 0),
                            stop=(sc == 7),
                        )
                # reciprocal of denominator row: custom-DVE + gpsimd bcast only
                # work at base partition 0 on HW, so recip all 33 lanes, DMA
                # lane 32 -> lane 0, then broadcast.
                rrow = smal.tile([33, TC], FP32, tag="rrow")
                nc.vector.reciprocal_approx_fast(rrow[:], po[:])
                r0 = smal.tile([1, TC], FP32, tag="r0")
                nc.gpsimd.dma_start(r0[:], rrow[32:33, :])
                rb = smal.tile([32, TC], FP32, tag="rb")
                nc.gpsimd.partition_broadcast(rb[:], r0[:])
                nc.vector.tensor_mul(o_norm[:, ts(t, TC)], po[0:32, :], rb[:])
                # final conv + fp32 residual
                py = ps_fin.tile([C, TC], FP32, tag="fin")
                nc.tensor.matmul(py[:], wo_sb[:], o_norm[:, ts(t, TC)])
                nc.vector.tensor_add(out_sb[:, ts(t, TC)], py[:], xf[:, ts(t, TC)])
                nc.sync.dma_start(out[i][:, ts(t, TC)], out_sb[:, ts(t, TC)])


# ---------------------------------------------------------------------------
# host-side driver
# ---------------------------------------------------------------------------

def _prep_consts(Wt, Wp, Wg, Wo, gamma):
    bf = ml_dtypes.bfloat16
    w_gp = np.zeros((C, 40), np.float32)
    w_gp[:, 0:32] = Wg.T
    w_gp[:, 32:40] = Wp.T
    w_tt = np.zeros((40, 2 * C), np.float32)
    w_tt[32:40, 0:C] = Wt
    w_tt[32:40, C : 2 * C] = Wt
    w_o = (np.float32(gamma) * Wo).T
    ident = np.eye(32, dtype=np.float32)
    return {
        "w_gp": w_gp.astype(bf),
        "w_tt": w_tt.astype(bf),
        "w_o": np.ascontiguousarray(w_o).astype(bf),
        "ident": ident.astype(bf),
    }


def kernel(x, Wt, Wp, Wg, Wo, gamma):
    x = np.asarray(x, dtype=np.float32)
    B = x.shape[0]
    n_cores = 8
    nper = B // n_cores
    xr = np.ascontiguousarray(x.reshape(B, C, HW))
    consts = _prep_consts(
        np.asarray(Wt, np.float32),
        np.asarray(Wp, np.float32),
        np.asarray(Wg, np.float32),
        np.asarray(Wo, np.float32),
        np.float32(gamma),
    )
    bf = ml_dtypes.bfloat16

    nc = build_nc(nper)
    in_maps = []
    for cid in range(n_cores):
        shard = xr[cid * nper : (cid + 1) * nper]
        in_maps.append(
            {
                "x_bf": shard.astype(bf),
                "x_f32": shard,
                **consts,
            }
        )
    res = run_bass_kernel_spmd(nc, in_maps, core_ids=list(range(n_cores)))
    outs = [res.results[cid]["out"] for cid in range(n_cores)]
    return np.concatenate(outs, axis=0).reshape(B, C, 64, 64)


# revision 28
# speedup vs baseline: 2.0301x; 1.1652x over previous
"""SAGAN-style attention block (B=16, C=64, H=W=64) on 8 trn2 NeuronCores.

Data-parallel over batch: 2 samples per core.  Per sample:
    g/phi conv -> 2x2 maxpool           (PE + DVE)
    phi~ = Wt^T @ phi                   (PE)   [folds theta away: scoresT = phi~^T x]
    scoresT[s,t] = phi~^T x  (s-chunks of 128 on partitions, t on free)
    expT = exp(scoresT)                 (ACT, no max-subtraction: |scores| < ~6)
    o_un[c,t], denom[t] = [g;1]^T-weighted matmul over s   (PE, ones-row trick)
    o_norm = o_un * (1/denom)           (DVE recip approx + GPSIMD partition bcast)
    y = (gamma*Wo) @ o_norm             (PE)
    out = y + x                         (DVE, fp32 residual)
"""

import numpy as np
import ml_dtypes

import concourse.bass as bass
import concourse.bacc as bacc
import concourse.tile as tile
from concourse import mybir
from concourse.bass_utils import run_bass_kernel_spmd

FP32 = mybir.dt.float32
BF16 = mybir.dt.bfloat16
I16 = mybir.dt.int16
SCH_A = float(128.0 / np.log(2.0))   # Schraudolph bf16: round(A*x+B) -> bf16 bits
SCH_B = 16256.0 - 5.6
SCHRAUDOLPH = False  # DVE-exp offload: off (ACT wasn't the span limiter)
ts = bass.ts

C = 64
HW = 4096          # 64*64
S = 1024           # pooled spatial (32*32)
NSAMP = 2          # samples per core
NCHUNK = 8         # both t-chunks (512 wide) and conv chunks
TC = 512           # t-chunk width
SC = 128           # s-chunk width


def build_nc(n_samples: int = NSAMP, repeat: int = 1) -> bass.Bass:
    nc = bacc.Bacc("TRN2", target_bir_lowering=False, debug=False)

    x_bf = nc.dram_tensor("x_bf", [n_samples, C, HW], BF16, kind="ExternalInput").ap()
    x_f32 = nc.dram_tensor("x_f32", [n_samples, C, HW], FP32, kind="ExternalInput").ap()
    # convA weights: columns 0-31 -> g channels (Wg), columns 32-39 -> phi (Wp)
    w_gp = nc.dram_tensor("w_gp", [C, 40], BF16, kind="ExternalInput").ap()
    # phi~ weights: rows 32-39 hold [Wt | Wt] [8, 128]; rows 0-31 zeros
    w_tt = nc.dram_tensor("w_tt", [40, 2 * C], BF16, kind="ExternalInput").ap()
    # final conv weights: (gamma*Wo)^T  [32, 64]
    w_o = nc.dram_tensor("w_o", [32, C], BF16, kind="ExternalInput").ap()
    ident = nc.dram_tensor("ident", [32, 32], BF16, kind="ExternalInput").ap()
    out = nc.dram_tensor("out", [n_samples, C, HW], FP32, kind="ExternalOutput").ap()

    with tile.TileContext(nc) as tc:
        for _ in range(repeat):
            _body(tc, n_samples, x_bf, x_f32, w_gp, w_tt, w_o, ident, out)
    nc.compile()
    return nc


def _body(tc, n_samples, x_bf, x_f32, w_gp, w_tt, w_o, ident, out):
    nc = tc.nc
    from contextlib import ExitStack

    with ExitStack() as ctx:
        consts = ctx.enter_context(tc.tile_pool(name="consts", bufs=1))
        xpool = ctx.enter_context(tc.tile_pool(name="xpool", bufs=2))
        mid = ctx.enter_context(tc.tile_pool(name="mid", bufs=2))
        expp = ctx.enter_context(tc.tile_pool(name="expp", bufs=6))
        smal = ctx.enter_context(tc.tile_pool(name="smal", bufs=6))
        outp = ctx.enter_context(tc.tile_pool(name="outp", bufs=2))
        ps_conv = ctx.enter_context(tc.tile_pool(name="ps_conv", bufs=1, space="PSUM"))
        ps_scorA = ctx.enter_context(tc.tile_pool(name="ps_scorA", bufs=1, space="PSUM"))
        ps_scorB = ctx.enter_context(tc.tile_pool(name="ps_scorB", bufs=1, space="PSUM"))
        ps_oacc = ctx.enter_context(tc.tile_pool(name="ps_oacc", bufs=2, space="PSUM"))
        ps_fin = ctx.enter_context(tc.tile_pool(name="ps_fin", bufs=1, space="PSUM"))

        # warm the ACT exp table set during setup (table load is ~2.7us)
        warm = consts.tile([1, 1], FP32)
        nc.vector.memset(warm[:], 0.0)
        nc.scalar.activation(warm[:], warm[:], mybir.ActivationFunctionType.Exp)

        wgp_sb = consts.tile([C, 40], BF16)
        nc.sync.dma_start(wgp_sb[:], w_gp[:])
        wtt_sb = consts.tile([40, 2 * C], BF16)
        nc.sync.dma_start(wtt_sb[:], w_tt[:])
        wo_sb = consts.tile([32, C], BF16)
        nc.sync.dma_start(wo_sb[:], w_o[:])
        id_sb = consts.tile([32, 32], BF16)
        nc.sync.dma_start(id_sb[:], ident[:])

        # ---- setup phase for every sample first (overlaps with attention of
        # earlier samples via scheduler priorities) ----------------------------
        setup = []
        for i in range(n_samples):
            xb = xpool.tile([2 * C, HW], BF16, tag="xb")
            xf = xpool.tile([C, HW], FP32, tag="xf")
            pooled = mid.tile([40, S], BF16, tag="pooled")
            phi2 = mid.tile([2 * C, S], BF16, tag="phi2")
            gT = mid.tile([SC, 33 * NCHUNK], BF16, tag="gT")

            # convA (g + phi) + 2x2 maxpool, per 512-col chunk; x replicated
            # to partitions 64-127 for 2-way row-packed scores matmuls
            for c in range(NCHUNK):
                nc.sync.dma_start(xb[0:C, ts(c, TC)], x_bf[i][:, ts(c, TC)])
                nc.sync.dma_start(xb[C : 2 * C, ts(c, TC)], x_bf[i][:, ts(c, TC)])
                nc.sync.dma_start(xf[:, ts(c, TC)], x_f32[i][:, ts(c, TC)])
                pa = ps_conv.tile([40, TC], FP32, tag="conv")
                nc.tensor.matmul(pa[:], wgp_sb[:], xb[0:C, ts(c, TC)])
                v = pa[:].rearrange("p (h eh w ew) -> p h w eh ew", h=4, eh=2, w=32, ew=2)
                pv = pooled[:, ts(c, SC)].rearrange("p (h w) -> p h w", h=4, w=32)
                nc.vector.tensor_reduce(
                    pv, v, axis=mybir.AxisListType.XY, op=mybir.AluOpType.max,
                    opt_input=False,
                )
                # phi~ for this s-chunk: [128, 128] (both replicas at once)
                ppt = ps_conv.tile([2 * C, SC], FP32, tag="conv")
                nc.tensor.matmul(
                    ppt[:], wtt_sb[32:40, :], pooled[32:40, ts(c, SC)]
                )
                nc.vector.tensor_copy(phi2[:, ts(c, SC)], ppt[:])

            # g'^T chunks [128, 33] with ones column
            ones_view = gT[:].rearrange("p (k c) -> p k c", k=NCHUNK, c=33)
            nc.vector.memset(ones_view[:, :, 32:33], 1.0)
            for k in range(NCHUNK):
                pt = ps_conv.tile([SC, 32], BF16, tag="conv")
                nc.tensor.transpose(pt[:], pooled[0:32, ts(k, SC)], id_sb[:])
                nc.vector.tensor_copy(gT[:, 33 * k : 33 * k + 32], pt[:])
            setup.append((xb, xf, phi2, gT))

        for i in range(n_samples):
            xb, xf, phi2, gT = setup[i]
            # ---- attention main loop over t-chunks ---------------------------
            o_norm = mid.tile([32, HW], BF16, tag="o_norm")
            out_sb = outp.tile([C, HW], FP32, tag="out_sb")
            for t in range(NCHUNK):
                po = ps_oacc.tile([33, TC], FP32, tag="oacc")
                for q in range(4):  # four groups of 2 s-chunks, alternating pools
                    pool_q = ps_scorA if q % 2 == 0 else ps_scorB
                    pscr = pool_q.tile([SC, 2 * TC], FP32, tag="scor")
                    # 2-way row-packed: s-chunk 2q on rows 0-63, 2q+1 on 64-127
                    nc.tensor.matmul(
                        pscr[:, ts(0, TC)],
                        phi2[0:C, ts(2 * q, SC)],
                        xb[0:C, ts(t, TC)],
                        tile_position=(0, 0),
                    )
                    nc.tensor.matmul(
                        pscr[:, ts(1, TC)],
                        phi2[C : 2 * C, ts(2 * q + 1, SC)],
                        xb[C : 2 * C, ts(t, TC)],
                        tile_position=(64, 0),
                    )
                    unit = (i * NCHUNK + t) * 4 + q
                    if SCHRAUDOLPH and unit % 7 == 3:
                        # Schraudolph exp on DVE: bf16 bit pattern via int16
                        e16 = expp.tile([SC, 2 * TC], I16, tag="expT")
                        nc.vector.tensor_scalar(
                            e16[:], pscr[:], SCH_A, SCH_B,
                            mybir.AluOpType.mult, mybir.AluOpType.add,
                        )
                        expT = e16[:].bitcast(BF16)
                    else:
                        et = expp.tile([SC, 2 * TC], BF16, tag="expT")
                        nc.scalar.activation(
                            et[:], pscr[:], mybir.ActivationFunctionType.Exp
                        )
                        expT = et[:]
                    for j in range(2):
                        sc = 2 * q + j
                        nc.tensor.matmul(
                            po[:],
                            gT[:, 33 * sc : 33 * sc + 33],
                            expT[:, ts(j, TC)],
                            start=(sc == 0),
                            stop=(sc == 7),
                        )
                # reciprocal of denominator row: custom-DVE + gpsimd bcast only
                # work at base partition 0 on HW, so recip all 33 lanes, DMA
                # lane 32 -> lane 0, then broadcast.
                rrow = smal.tile([33, TC], FP32, tag="rrow")
                nc.vector.reciprocal_approx_fast(rrow[:], po[:])
                r0 = smal.tile([1, TC], FP32, tag="r0")
                nc.gpsimd.dma_start(r0[:], rrow[32:33, :])
                rb = smal.tile([32, TC], FP32, tag="rb")
                nc.gpsimd.partition_broadcast(rb[:], r0[:])
                nc.vector.tensor_mul(o_norm[:, ts(t, TC)], po[0:32, :], rb[:])
                # final conv + fp32 residual
                py = ps_fin.tile([C, TC], FP32, tag="fin")
                nc.tensor.matmul(py[:], wo_sb[:], o_norm[:, ts(t, TC)])
                nc.vector.tensor_add(out_sb[:, ts(t, TC)], py[:], xf[:, ts(t, TC)])
                nc.sync.dma_start(out[i][:, ts(t, TC)], out_sb[:, ts(t, TC)])


# ---------------------------------------------------------------------------
# host-side driver
# ---------------------------------------------------------------------------

def _prep_consts(Wt, Wp, Wg, Wo, gamma):
    bf = ml_dtypes.bfloat16
    w_gp = np.zeros((C, 40), np.float32)
    w_gp[:, 0:32] = Wg.T
    w_gp[:, 32:40] = Wp.T
    w_tt = np.zeros((40, 2 * C), np.float32)
    w_tt[32:40, 0:C] = Wt
    w_tt[32:40, C : 2 * C] = Wt
    w_o = (np.float32(gamma) * Wo).T
    ident = np.eye(32, dtype=np.float32)
    return {
        "w_gp": w_gp.astype(bf),
        "w_tt": w_tt.astype(bf),
        "w_o": np.ascontiguousarray(w_o).astype(bf),
        "ident": ident.astype(bf),
    }


def kernel(x, Wt, Wp, Wg, Wo, gamma):
    x = np.asarray(x, dtype=np.float32)
    B = x.shape[0]
    n_cores = 8
    nper = B // n_cores
    xr = np.ascontiguousarray(x.reshape(B, C, HW))
    consts = _prep_consts(
        np.asarray(Wt, np.float32),
        np.asarray(Wp, np.float32),
        np.asarray(Wg, np.float32),
        np.asarray(Wo, np.float32),
        np.float32(gamma),
    )
    bf = ml_dtypes.bfloat16

    nc = build_nc(nper)
    in_maps = []
    for cid in range(n_cores):
        shard = xr[cid * nper : (cid + 1) * nper]
        in_maps.append(
            {
                "x_bf": shard.astype(bf),
                "x_f32": shard,
                **consts,
            }
        )
    res = run_bass_kernel_spmd(nc, in_maps, core_ids=list(range(n_cores)))
    outs = [res.results[cid]["out"] for cid in range(n_cores)]
    return np.concatenate(outs, axis=0).reshape(B, C, 64, 64)


# revision 37
# speedup vs baseline: 2.6484x; 1.3046x over previous
"""SAGAN-style attention block (B=16, C=64, H=W=64) on 8 trn2 NeuronCores.

Data-parallel over batch: 2 samples per core.  Per sample:
    g/phi conv -> 2x2 maxpool           (PE + DVE)
    phi~ = Wt^T @ phi                   (PE)   [folds theta away: scoresT = phi~^T x]
    scoresT[s,t] = phi~^T x  (s-chunks of 128 on partitions, t on free)
    expT = exp(scoresT)                 (ACT, no max-subtraction: |scores| < ~6)
    o_un[c,t], denom[t] = [g;1]^T-weighted matmul over s   (PE, ones-row trick)
    o_norm = o_un * (1/denom)           (DVE recip approx + GPSIMD partition bcast)
    y = (gamma*Wo) @ o_norm             (PE)
    out = y + x                         (DVE, fp32 residual)
"""

import numpy as np
import ml_dtypes

import concourse.bass as bass
import concourse.bacc as bacc
import concourse.tile as tile
from concourse import mybir
from concourse.bass_utils import run_bass_kernel_spmd

FP32 = mybir.dt.float32
BF16 = mybir.dt.bfloat16
I16 = mybir.dt.int16
SCH_A = float(128.0 / np.log(2.0))   # Schraudolph bf16: round(A*x+B) -> bf16 bits
SCH_B = 16256.0 - 5.6
SCHRAUDOLPH = True   # one quarter every other chunk on DVE
ts = bass.ts

C = 64
HW = 4096          # 64*64
S = 1024           # pooled spatial (32*32)
NSAMP = 2          # samples per core
NCHUNK = 8         # both t-chunks (512 wide) and conv chunks
TC = 512           # t-chunk width
SC = 128           # s-chunk width


def build_nc(n_samples: int = NSAMP, repeat: int = 1) -> bass.Bass:
    nc = bacc.Bacc("TRN2", target_bir_lowering=False, debug=False)

    x_bf = nc.dram_tensor("x_bf", [n_samples, C, HW], BF16, kind="ExternalInput").ap()
    x_f32 = nc.dram_tensor("x_f32", [n_samples, C, HW], FP32, kind="ExternalInput").ap()
    # convA weights: columns 0-31 -> g channels (Wg), columns 32-39 -> phi (Wp)
    w_gp = nc.dram_tensor("w_gp", [C, 40], BF16, kind="ExternalInput").ap()
    # phi~ weights: rows 32-39 hold [Wt | Wt] [8, 128]; rows 0-31 zeros
    w_tt = nc.dram_tensor("w_tt", [40, 2 * C], BF16, kind="ExternalInput").ap()
    # final conv weights: (gamma*Wo)^T  [32, 64]
    w_o = nc.dram_tensor("w_o", [32, C], BF16, kind="ExternalInput").ap()
    ident = nc.dram_tensor("ident", [32, 32], BF16, kind="ExternalInput").ap()
    out = nc.dram_tensor("out", [n_samples, C, HW], FP32, kind="ExternalOutput").ap()

    with tile.TileContext(nc) as tc:
        for _ in range(repeat):
            _body(tc, n_samples, x_bf, x_f32, w_gp, w_tt, w_o, ident, out)
    nc.compile()
    return nc


def _body(tc, n_samples, x_bf, x_f32, w_gp, w_tt, w_o, ident, out):
    nc = tc.nc
    from contextlib import ExitStack

    with ExitStack() as ctx:
        consts = ctx.enter_context(tc.tile_pool(name="consts", bufs=1))
        xpool = ctx.enter_context(tc.tile_pool(name="xpool", bufs=2))
        mid = ctx.enter_context(tc.tile_pool(name="mid", bufs=2))
        expp = ctx.enter_context(tc.tile_pool(name="expp", bufs=9))
        smal = ctx.enter_context(tc.tile_pool(name="smal", bufs=6))
        outp = ctx.enter_context(tc.tile_pool(name="outp", bufs=2))
        ps_conv = ctx.enter_context(tc.tile_pool(name="ps_conv", bufs=1, space="PSUM"))
        ps_scorA = ctx.enter_context(tc.tile_pool(name="ps_scorA", bufs=1, space="PSUM"))
        ps_scorB = ctx.enter_context(tc.tile_pool(name="ps_scorB", bufs=1, space="PSUM"))
        ps_oacc = ctx.enter_context(tc.tile_pool(name="ps_oacc", bufs=2, space="PSUM"))
        ps_fin = ctx.enter_context(tc.tile_pool(name="ps_fin", bufs=1, space="PSUM"))

        # warm the ACT exp table set during setup (table load is ~2.7us)
        warm = consts.tile([1, 1], FP32)
        nc.vector.memset(warm[:], 0.0)
        nc.scalar.activation(warm[:], warm[:], mybir.ActivationFunctionType.Exp)

        wgp_sb = consts.tile([C, 40], BF16)
        nc.sync.dma_start(wgp_sb[:], w_gp[:])
        wtt_sb = consts.tile([40, 2 * C], BF16)
        wo_sb = consts.tile([32, C], BF16)
        id_sb = consts.tile([32, 32], BF16)
        nc.gpsimd.dma_start(wtt_sb[:], w_tt[:])
        nc.gpsimd.dma_start(wo_sb[:], w_o[:])
        nc.gpsimd.dma_start(id_sb[:], ident[:])

        # ---- setup phase for every sample first (overlaps with attention of
        # earlier samples via scheduler priorities) ----------------------------
        setup = []
        for i in range(n_samples):
            xb = xpool.tile([2 * C, HW], BF16, tag="xb")
            xf = xpool.tile([C, HW], FP32, tag="xf")
            pooled = mid.tile([40, S], BF16, tag="pooled")
            phi2 = mid.tile([2 * C, S], BF16, tag="phi2")
            gT = mid.tile([SC, 33 * NCHUNK], BF16, tag="gT")

            # convA (g + phi) + 2x2 maxpool, per 512-col chunk; x replicated
            # to partitions 64-127 for 2-way row-packed scores matmuls
            for c in range(NCHUNK):
                nc.sync.dma_start(xb[0:C, ts(c, TC)], x_bf[i][:, ts(c, TC)])
                nc.gpsimd.dma_start(xb[C : 2 * C, ts(c, TC)], x_bf[i][:, ts(c, TC)])
                nc.gpsimd.dma_start(xf[:, ts(c, TC)], x_f32[i][:, ts(c, TC)])
                pa = ps_conv.tile([40, TC], FP32, tag="conv")
                nc.tensor.matmul(pa[:], wgp_sb[:], xb[0:C, ts(c, TC)])
                v = pa[:].rearrange("p (h eh w ew) -> p h w eh ew", h=4, eh=2, w=32, ew=2)
                pv = pooled[:, ts(c, SC)].rearrange("p (h w) -> p h w", h=4, w=32)
                nc.vector.tensor_reduce(
                    pv, v, axis=mybir.AxisListType.XY, op=mybir.AluOpType.max,
                    opt_input=False,
                )
                # phi~ for this s-chunk: [128, 128] (both replicas at once)
                # (fin pool is idle during setup; avoids serializing on the
                # conv slot between pa and ppt)
                ppt = ps_fin.tile([2 * C, SC], FP32, tag="fin")
                nc.tensor.matmul(
                    ppt[:], wtt_sb[32:40, :], pooled[32:40, ts(c, SC)]
                )
                nc.vector.tensor_copy(phi2[:, ts(c, SC)], ppt[:])

            # g'^T chunks [128, 33] with ones column
            ones_view = gT[:].rearrange("p (k c) -> p k c", k=NCHUNK, c=33)
            nc.vector.memset(ones_view[:, :, 32:33], 1.0)
            for k in range(NCHUNK):
                pt = ps_conv.tile([SC, 32], BF16, tag="conv")
                nc.tensor.transpose(pt[:], pooled[0:32, ts(k, SC)], id_sb[:])
                nc.vector.tensor_copy(gT[:, 33 * k : 33 * k + 32], pt[:])
            setup.append((xb, xf, phi2, gT))

        for i in range(n_samples):
            xb, xf, phi2, gT = setup[i]
            # ---- attention main loop over t-chunks, software-pipelined:
            # emit scores+exp for chunk t, but o'+tail for chunk t-1, so a
            # po-slot stall never head-of-line-blocks the next scores on PE.
            o_norm = mid.tile([32, HW], BF16, tag="o_norm")
            out_sb = outp.tile([C, HW], FP32, tag="out_sb")
            pending = None  # (t, expT list)

            def emit_scores(t):
                exps = []
                for q in range(4):
                    pool_q = ps_scorA if q % 2 == 0 else ps_scorB
                    pscr = pool_q.tile([SC, 2 * TC], FP32, tag="scor")
                    nc.tensor.matmul(
                        pscr[:, ts(0, TC)],
                        phi2[0:C, ts(2 * q, SC)],
                        xb[0:C, ts(t, TC)],
                        tile_position=(0, 0),
                    )
                    nc.tensor.matmul(
                        pscr[:, ts(1, TC)],
                        phi2[C : 2 * C, ts(2 * q + 1, SC)],
                        xb[C : 2 * C, ts(t, TC)],
                        tile_position=(64, 0),
                    )
                    if SCHRAUDOLPH and q == 1 and t % 2 == 1:
                        e16 = expp.tile([SC, 2 * TC], I16, tag="expT")
                        nc.vector.tensor_scalar(
                            e16[:], pscr[:], SCH_A, SCH_B,
                            mybir.AluOpType.mult, mybir.AluOpType.add,
                        )
                        exps.append(e16[:].bitcast(BF16))
                    else:
                        et = expp.tile([SC, 2 * TC], BF16, tag="expT")
                        nc.scalar.activation(
                            et[:], pscr[:], mybir.ActivationFunctionType.Exp
                        )
                        exps.append(et[:])
                return exps

            def emit_ovalue(t, exps):
                po = ps_oacc.tile([33, TC], FP32, tag="oacc")
                for q in range(4):
                    for j in range(2):
                        sc = 2 * q + j
                        nc.tensor.matmul(
                            po[:],
                            gT[:, 33 * sc : 33 * sc + 33],
                            exps[q][:, ts(j, TC)],
                            start=(sc == 0),
                            stop=(sc == 7),
                        )
                # recip of denominator row: custom-DVE + gpsimd bcast only work
                # at base partition 0 on HW -> recip all 33 lanes, DMA lane 32
                # to lane 0, then broadcast.
                rrow = smal.tile([33, TC], FP32, tag="rrow")
                nc.vector.reciprocal_approx_fast(rrow[:], po[:])
                r0 = smal.tile([1, TC], FP32, tag="r0")
                nc.gpsimd.dma_start(r0[:], rrow[32:33, :])
                rb = smal.tile([32, TC], FP32, tag="rb")
                nc.gpsimd.partition_broadcast(rb[:], r0[:])
                nc.vector.tensor_mul(o_norm[:, ts(t, TC)], po[0:32, :], rb[:])
                py = ps_fin.tile([C, TC], FP32, tag="fin")
                nc.tensor.matmul(py[:], wo_sb[:], o_norm[:, ts(t, TC)])
                nc.vector.tensor_add(out_sb[:, ts(t, TC)], py[:], xf[:, ts(t, TC)])
                nc.sync.dma_start(out[i][:, ts(t, TC)], out_sb[:, ts(t, TC)])

            for t in range(NCHUNK):
                exps = emit_scores(t)
                if pending is not None:
                    emit_ovalue(*pending)
                if t == NCHUNK - 1:
                    emit_ovalue(t, exps)
                    pending = None
                else:
                    pending = (t, exps)


# ---------------------------------------------------------------------------
# host-side driver
# ---------------------------------------------------------------------------

def _prep_consts(Wt, Wp, Wg, Wo, gamma):
    bf = ml_dtypes.bfloat16
    w_gp = np.zeros((C, 40), np.float32)
    w_gp[:, 0:32] = Wg.T
    w_gp[:, 32:40] = Wp.T
    w_tt = np.zeros((40, 2 * C), np.float32)
    w_tt[32:40, 0:C] = Wt
    w_tt[32:40, C : 2 * C] = Wt
    w_o = (np.float32(gamma) * Wo).T
    ident = np.eye(32, dtype=np.float32)
    return {
        "w_gp": w_gp.astype(bf),
        "w_tt": w_tt.astype(bf),
        "w_o": np.ascontiguousarray(w_o).astype(bf),
        "ident": ident.astype(bf),
    }


def kernel(x, Wt, Wp, Wg, Wo, gamma):
    x = np.asarray(x, dtype=np.float32)
    B = x.shape[0]
    n_cores = 8
    nper = B // n_cores
    xr = np.ascontiguousarray(x.reshape(B, C, HW))
    consts = _prep_consts(
        np.asarray(Wt, np.float32),
        np.asarray(Wp, np.float32),
        np.asarray(Wg, np.float32),
        np.asarray(Wo, np.float32),
        np.float32(gamma),
    )
    bf = ml_dtypes.bfloat16

    nc = build_nc(nper)
    in_maps = []
    for cid in range(n_cores):
        shard = xr[cid * nper : (cid + 1) * nper]
        in_maps.append(
            {
                "x_bf": shard.astype(bf),
                "x_f32": shard,
                **consts,
            }
        )
    res = run_bass_kernel_spmd(nc, in_maps, core_ids=list(range(n_cores)))
    outs = [res.results[cid]["out"] for cid in range(n_cores)]
    return np.concatenate(outs, axis=0).reshape(B, C, 64, 64)


# revision 47
# speedup vs baseline: 3.1316x; 1.1825x over previous
"""SAGAN-style attention block (B=16, C=64, H=W=64) on 8 trn2 NeuronCores.

Data-parallel over batch: 2 samples per core.  Per sample:
    g/phi conv -> 2x2 maxpool           (PE + DVE)
    phi~ = Wt^T @ phi                   (PE)   [folds theta away: scoresT = phi~^T x]
    scoresT[s,t] = phi~^T x  (s-chunks of 128 on partitions, t on free)
    expT = exp(scoresT)                 (ACT, no max-subtraction: |scores| < ~6)
    o_un[c,t], denom[t] = [g;1]^T-weighted matmul over s   (PE, ones-row trick)
    o_norm = o_un * (1/denom)           (DVE recip approx + GPSIMD partition bcast)
    y = (gamma*Wo) @ o_norm             (PE)
    out = y + x                         (DVE, fp32 residual)
"""

import numpy as np
import ml_dtypes

import concourse.bass as bass
import concourse.bacc as bacc
import concourse.tile as tile
from concourse import mybir
from concourse.bass_utils import run_bass_kernel_spmd

FP32 = mybir.dt.float32
BF16 = mybir.dt.bfloat16
I16 = mybir.dt.int16
SCH_A = float(128.0 / np.log(2.0))   # Schraudolph bf16: round(A*x+B) -> bf16 bits
SCH_B = 16256.0 - 5.6
SCHRAUDOLPH = True   # one quarter every other chunk on DVE
ts = bass.ts

C = 64
HW = 4096          # 64*64
S = 1024           # pooled spatial (32*32)
NSAMP = 2          # samples per core
NCHUNK = 8         # both t-chunks (512 wide) and conv chunks
TC = 512           # t-chunk width
SC = 128           # s-chunk width


def build_nc(n_samples: int = NSAMP, repeat: int = 1) -> bass.Bass:
    nc = bacc.Bacc("TRN2", target_bir_lowering=False, debug=False)

    x_bf = nc.dram_tensor("x_bf", [n_samples, C, HW], BF16, kind="ExternalInput").ap()
    x_f32 = nc.dram_tensor("x_f32", [n_samples, C, HW], FP32, kind="ExternalInput").ap()
    # convA weights: columns 0-31 -> g channels (Wg), columns 32-39 -> phi (Wp)
    w_gp = nc.dram_tensor("w_gp", [C, 40], BF16, kind="ExternalInput").ap()
    # phi~ weights: rows 32-39 hold [Wt | Wt] [8, 128]; rows 0-31 zeros
    w_tt = nc.dram_tensor("w_tt", [40, 2 * C], BF16, kind="ExternalInput").ap()
    # final conv weights: (gamma*Wo)^T  [32, 64]
    w_o = nc.dram_tensor("w_o", [32, C], BF16, kind="ExternalInput").ap()
    ident = nc.dram_tensor("ident", [32, 32], BF16, kind="ExternalInput").ap()
    out = nc.dram_tensor("out", [n_samples, C, HW], FP32, kind="ExternalOutput").ap()

    with tile.TileContext(nc) as tc:
        for _ in range(repeat):
            _body(tc, n_samples, x_bf, x_f32, w_gp, w_tt, w_o, ident, out)
    nc.compile()
    return nc


def _body(tc, n_samples, x_bf, x_f32, w_gp, w_tt, w_o, ident, out):
    nc = tc.nc
    from contextlib import ExitStack

    with ExitStack() as ctx:
        consts = ctx.enter_context(tc.tile_pool(name="consts", bufs=1))
        xpool = ctx.enter_context(tc.tile_pool(name="xpool", bufs=2))
        mid = ctx.enter_context(tc.tile_pool(name="mid", bufs=2))
        expp = ctx.enter_context(tc.tile_pool(name="expp", bufs=9))
        smal = ctx.enter_context(tc.tile_pool(name="smal", bufs=6))
        outp = ctx.enter_context(tc.tile_pool(name="outp", bufs=2))
        ps_conv = ctx.enter_context(tc.tile_pool(name="ps_conv", bufs=1, space="PSUM"))
        ps_scorA = ctx.enter_context(tc.tile_pool(name="ps_scorA", bufs=1, space="PSUM"))
        ps_scorB = ctx.enter_context(tc.tile_pool(name="ps_scorB", bufs=1, space="PSUM"))
        ps_oacc = ctx.enter_context(tc.tile_pool(name="ps_oacc", bufs=2, space="PSUM"))
        ps_fin = ctx.enter_context(tc.tile_pool(name="ps_fin", bufs=1, space="PSUM"))

        # warm the ACT exp table set during setup (table load is ~2.7us)
        warm = consts.tile([1, 1], FP32)
        nc.vector.memset(warm[:], 0.0)
        nc.scalar.activation(warm[:], warm[:], mybir.ActivationFunctionType.Exp)

        wgp_sb = consts.tile([C, 40], BF16)
        nc.sync.dma_start(wgp_sb[:], w_gp[:])
        wtt_sb = consts.tile([40, 2 * C], BF16)
        wo_sb = consts.tile([32, C], BF16)
        id_sb = consts.tile([32, 32], BF16)
        nc.gpsimd.dma_start(wtt_sb[:], w_tt[:])
        nc.gpsimd.dma_start(wo_sb[:], w_o[:])
        nc.gpsimd.dma_start(id_sb[:], ident[:])

        # ---- setup phase for every sample first (overlaps with attention of
        # earlier samples via scheduler priorities) ----------------------------
        setup = []
        for i in range(n_samples):
            xb = xpool.tile([2 * C, HW], BF16, tag="xb")
            xf = xpool.tile([C, HW], FP32, tag="xf")
            pooled = mid.tile([40, S], BF16, tag="pooled")
            phi2 = mid.tile([2 * C, S], BF16, tag="phi2")
            gT = mid.tile([SC, 33 * NCHUNK], BF16, tag="gT")

            # convA (g + phi) + 2x2 maxpool, per 512-col chunk; x replicated
            # to partitions 64-127 for 2-way row-packed scores matmuls
            for c in range(NCHUNK):
                nc.sync.dma_start(xb[0:C, ts(c, TC)], x_bf[i][:, ts(c, TC)])
                nc.gpsimd.dma_start(xb[C : 2 * C, ts(c, TC)], x_bf[i][:, ts(c, TC)])
                nc.gpsimd.dma_start(xf[:, ts(c, TC)], x_f32[i][:, ts(c, TC)])
                pa = ps_conv.tile([40, TC], FP32, tag="conv")
                nc.tensor.matmul(pa[:], wgp_sb[:], xb[0:C, ts(c, TC)])
                v = pa[:].rearrange("p (h eh w ew) -> p h w eh ew", h=4, eh=2, w=32, ew=2)
                pv = pooled[:, ts(c, SC)].rearrange("p (h w) -> p h w", h=4, w=32)
                nc.vector.tensor_reduce(
                    pv, v, axis=mybir.AxisListType.XY, op=mybir.AluOpType.max,
                    opt_input=False,
                )
                # phi~ for this s-chunk: [128, 128] (both replicas at once)
                # (fin pool is idle during setup; avoids serializing on the
                # conv slot between pa and ppt)
                ppt = ps_fin.tile([2 * C, SC], FP32, tag="fin")
                nc.tensor.matmul(
                    ppt[:], wtt_sb[32:40, :], pooled[32:40, ts(c, SC)]
                )
                nc.vector.tensor_copy(phi2[:, ts(c, SC)], ppt[:])

            # g'^T chunks [128, 33] with ones column
            ones_view = gT[:].rearrange("p (k c) -> p k c", k=NCHUNK, c=33)
            nc.vector.memset(ones_view[:, :, 32:33], 1.0)
            for k in range(NCHUNK):
                pt = ps_conv.tile([SC, 32], BF16, tag="conv")
                nc.tensor.transpose(pt[:], pooled[0:32, ts(k, SC)], id_sb[:])
                nc.vector.tensor_copy(gT[:, 33 * k : 33 * k + 32], pt[:])
            setup.append((xb, xf, phi2, gT))

        for i in range(n_samples):
            xb, xf, phi2, gT = setup[i]
            # ---- attention main loop over t-chunks, software-pipelined:
            # emit scores+exp for chunk t, but o'+tail for chunk t-1, so a
            # po-slot stall never head-of-line-blocks the next scores on PE.
            o_norm = mid.tile([32, HW], BF16, tag="o_norm")
            out_sb = outp.tile([C, HW], FP32, tag="out_sb")
            pending = None  # (t, expT list)

            def emit_scores(t, order=None, exps=None):
                exps = [None] * 4 if exps is None else exps
                for q in (order or range(4)):
                    pool_q = ps_scorA if q % 2 == 0 else ps_scorB
                    pscr = pool_q.tile([SC, 2 * TC], FP32, tag="scor")
                    nc.tensor.matmul(
                        pscr[:, ts(0, TC)],
                        phi2[0:C, ts(2 * q, SC)],
                        xb[0:C, ts(t, TC)],
                        tile_position=(0, 0),
                    )
                    nc.tensor.matmul(
                        pscr[:, ts(1, TC)],
                        phi2[C : 2 * C, ts(2 * q + 1, SC)],
                        xb[C : 2 * C, ts(t, TC)],
                        tile_position=(64, 0),
                    )
                    if SCHRAUDOLPH and q == 1 and t % 2 == 1:
                        e16 = expp.tile([SC, 2 * TC], I16, tag="expT")
                        nc.vector.tensor_scalar(
                            e16[:], pscr[:], SCH_A, SCH_B,
                            mybir.AluOpType.mult, mybir.AluOpType.add,
                        )
                        exps[q] = e16[:].bitcast(BF16)
                    else:
                        et = expp.tile([SC, 2 * TC], BF16, tag="expT")
                        nc.scalar.activation(
                            et[:], pscr[:], mybir.ActivationFunctionType.Exp
                        )
                        exps[q] = et[:]
                return exps

            def emit_ovalue(t, exps):
                po = ps_oacc.tile([33, TC], FP32, tag="oacc")
                for q in range(4):
                    for j in range(2):
                        sc = 2 * q + j
                        nc.tensor.matmul(
                            po[:],
                            gT[:, 33 * sc : 33 * sc + 33],
                            exps[q][:, ts(j, TC)],
                            start=(sc == 0),
                            stop=(sc == 7),
                        )
                # recip of denominator row: custom-DVE + gpsimd bcast only work
                # at base partition 0 on HW -> recip all 33 lanes, DMA lane 32
                # to lane 0, then broadcast.
                rrow = smal.tile([33, TC], FP32, tag="rrow")
                nc.vector.reciprocal_approx_fast(rrow[:], po[:])
                r0 = smal.tile([1, TC], FP32, tag="r0")
                nc.gpsimd.dma_start(r0[:], rrow[32:33, :])
                rb = smal.tile([32, TC], FP32, tag="rb")
                nc.gpsimd.partition_broadcast(rb[:], r0[:])
                nc.vector.tensor_mul(o_norm[:, ts(t, TC)], po[0:32, :], rb[:])
                py = ps_fin.tile([C, TC], FP32, tag="fin")
                nc.tensor.matmul(py[:], wo_sb[:], o_norm[:, ts(t, TC)])
                nc.vector.tensor_add(out_sb[:, ts(t, TC)], py[:], xf[:, ts(t, TC)])
                nc.sync.dma_start(out[i][:, ts(t, TC)], out_sb[:, ts(t, TC)])

            for t in range(NCHUNK):
                exps = emit_scores(t)
                if pending is not None:
                    emit_ovalue(*pending)
                if t == NCHUNK - 1:
                    emit_ovalue(t, exps)
                    pending = None
                else:
                    pending = (t, exps)


# ---------------------------------------------------------------------------
# host-side driver
# ---------------------------------------------------------------------------

def _prep_consts(Wt, Wp, Wg, Wo, gamma):
    bf = ml_dtypes.bfloat16
    w_gp = np.zeros((C, 40), np.float32)
    w_gp[:, 0:32] = Wg.T
    w_gp[:, 32:40] = Wp.T
    w_tt = np.zeros((40, 2 * C), np.float32)
    w_tt[32:40, 0:C] = Wt
    w_tt[32:40, C : 2 * C] = Wt
    w_o = (np.float32(gamma) * Wo).T
    ident = np.eye(32, dtype=np.float32)
    return {
        "w_gp": w_gp.astype(bf),
        "w_tt": w_tt.astype(bf),
        "w_o": np.ascontiguousarray(w_o).astype(bf),
        "ident": ident.astype(bf),
    }


def kernel(x, Wt, Wp, Wg, Wo, gamma):
    x = np.asarray(x, dtype=np.float32)
    B = x.shape[0]
    n_cores = 8
    nper = B // n_cores
    xr = np.ascontiguousarray(x.reshape(B, C, HW))
    consts = _prep_consts(
        np.asarray(Wt, np.float32),
        np.asarray(Wp, np.float32),
        np.asarray(Wg, np.float32),
        np.asarray(Wo, np.float32),
        np.float32(gamma),
    )
    bf = ml_dtypes.bfloat16

    nc = build_nc(nper)
    in_maps = []
    for cid in range(n_cores):
        shard = xr[cid * nper : (cid + 1) * nper]
        in_maps.append(
            {
                "x_bf": shard.astype(bf),
                "x_f32": shard,
                **consts,
            }
        )
    res = run_bass_kernel_spmd(nc, in_maps, core_ids=list(range(n_cores)))
    outs = [res.results[cid]["out"] for cid in range(n_cores)]
    return np.concatenate(outs, axis=0).reshape(B, C, 64, 64)
